# revision 6
# baseline (speedup 1.0000x reference)
"""Trainium2 Bass kernel for nn_ConvIntrinsicLite.

Math notes
----------
The reference computes, per vertex n:
  pts[n,k,f]   = sum_j bw[n,k,j] * mesh[idx[n,k,j], f]          (k in 0..39 = (r,a))
  interp       = einsum('nkf,rak->nraf', pts, coef)
  out[n,s,:]   = sum_K relu( W @ sum_{r,a} roll(interp, s, a) + bias )

A circular roll along `a` followed by the full sum over (r,a) is roll-invariant,
so every rotation s yields identical values.  The (r,a) sum of interp collapses
onto csum[k] = sum_{r,a} coef[r,a,k]:
  S[n,f] = sum_{t=(k,j)} csum[k]*bw[n,t] * mesh[idx[n,t], f]    (120 terms)
  z[n,(K,o)] = S[n,:] @ Wfc + bias ;  out0 = relu(z)[:, :64] + relu(z)[:, 64:]
  out[n,s,:] = out0[n,:] for all 8 rotations.

Implementation split
--------------------
The environment's data-dependent DMA paths (anthropic extended dma_gather
ucode, multi-offset indirect DMA) are broken/disabled here, so the host
resolves the irregular row lookup G[n,t,:] = mesh_signal[idx[n,t],:] (pure
data movement).  All arithmetic runs on device, sharded over vertices on
8 cores:
  DVE: v = csum*bw, prod = G*v, S = sum_t prod
  PE : S^T via identity transpose, z = S @ Wfc
  DVE: +bias, relu, fold K pair, replicate 8 rotations; DMA out.
"""

import sys

import numpy as np

for _p in ("/opt/trn_rl_repo",):
    if _p not in sys.path:
        sys.path.insert(0, _p)

# problem dims
N, R, A, F = 25000, 5, 8, 32
K, O = 2, 64
NROT = 8            # rotations = range(0, A, 1)
T = R * A * 3       # 120 weighted terms per vertex
NCORES = 8
P = 128
NV = N // NCORES            # 3125 vertices per core
TPC = (NV + P - 1) // P     # 25 tiles per core
NVPAD = TPC * P             # 3200

_CACHE = {}


def _build_module(tpc, loops=1, variant="full"):
    import concourse.bacc as bacc
    import concourse.mybir as mybir
    from concourse.tile import TileContext

    f32 = mybir.dt.float32

    nc = bacc.Bacc("TRN2", target_bir_lowering=False, debug=False)

    # gathered mesh rows, feature-major per vertex: gw[n, f, t]
    gw = nc.dram_tensor("gw", [tpc, P, F, T], f32, kind="ExternalInput")
    # barycentric weights bw[n, t]
    bwt = nc.dram_tensor("bwt", [tpc, P, T], f32, kind="ExternalInput")
    # csum[k] replicated over j: cs3[t] = csum[t // 3], as a [1, T] row
    cs3 = nc.dram_tensor("cs3", [1, T], f32, kind="ExternalInput")
    wfc = nc.dram_tensor("wfc", [F, K * O], f32, kind="ExternalInput")
    biasr = nc.dram_tensor("biasr", [1, K * O], f32, kind="ExternalInput")
    ident = nc.dram_tensor("ident", [P, P], f32, kind="ExternalInput")
    out = nc.dram_tensor("out", [tpc * P, NROT, O], f32, kind="ExternalOutput")

    mult = mybir.AluOpType.mult
    add = mybir.AluOpType.add

    with TileContext(nc) as tc:
        with (
            tc.tile_pool(name="const", bufs=1) as cpool,
            tc.tile_pool(name="io", bufs=3) as iopool,
            tc.tile_pool(name="gath", bufs=3) as gpool,
            tc.tile_pool(name="work", bufs=2) as wpool,
            tc.tile_pool(name="psum", bufs=2, space="PSUM") as ppool,
        ):
            wfc_sb = cpool.tile([F, K * O], f32)
            nc.sync.dma_start(out=wfc_sb[:], in_=wfc[:])
            bias_sb = cpool.tile([P, K * O], f32)
            nc.sync.dma_start(out=bias_sb[:], in_=biasr[:].to_broadcast((P, K * O)))
            ident_sb = cpool.tile([P, P], f32)
            nc.sync.dma_start(out=ident_sb[:], in_=ident[:])
            cs_sb = cpool.tile([P, T], f32)
            nc.sync.dma_start(out=cs_sb[:], in_=cs3[:].to_broadcast((P, T)))

            import contextlib

            loop_ctx = (
                tc.For_i(0, loops, 1) if loops > 1 else contextlib.nullcontext()
            )
            with loop_ctx:
              for t in range(tpc):
                g_sb = gpool.tile([P, F, T], f32, tag="g")
                nc.sync.dma_start(out=g_sb[:], in_=gw[t])
                bw_sb = iopool.tile([P, T], f32, tag="bw")
                nc.sync.dma_start(out=bw_sb[:], in_=bwt[t])

                if variant == "dmaonly":
                    rep0 = wpool.tile([P, NROT, O], f32, tag="rep")
                    nc.vector.tensor_copy(
                        out=rep0[:],
                        in_=g_sb[:, 0, 0:O]
                        .rearrange("p (r o) -> p r o", r=1)
                        .to_broadcast((P, NROT, O)),
                    )
                    nc.sync.dma_start(
                        out=out[t * P : (t + 1) * P, :, :], in_=rep0[:]
                    )
                    continue

                # v[p, t] = bw[p, t] * csum3[t]
                v_sb = wpool.tile([P, T], f32, tag="v")
                nc.vector.tensor_tensor(
                    out=v_sb[:], in0=bw_sb[:], in1=cs_sb[:], op=mult
                )
                # prod[p, f, t] = g[p, f, t] * v[p, t]
                prod = wpool.tile([P, F, T], f32, tag="prod")
                nc.vector.tensor_tensor(
                    out=prod[:],
                    in0=g_sb[:],
                    in1=v_sb[:]
                    .rearrange("p (o t) -> p o t", o=1)
                    .to_broadcast((P, F, T)),
                    op=mult,
                )
                # S[p, f] = sum_t prod[p, f, t]
                s_sb = wpool.tile([P, F], f32, tag="s")
                nc.vector.tensor_reduce(
                    out=s_sb[:], in_=prod[:], axis=mybir.AxisListType.X, op=add
                )

                st_ps = ppool.tile([F, P], f32, tag="stp")
                nc.tensor.transpose(out=st_ps[:], in_=s_sb[:], identity=ident_sb[:])
                st_sb = wpool.tile([F, P], f32, tag="st")
                nc.vector.tensor_copy(out=st_sb[:], in_=st_ps[:])

                z_ps = ppool.tile([P, K * O], f32, tag="z")
                nc.tensor.matmul(
                    out=z_ps[:], lhsT=st_sb[:], rhs=wfc_sb[:], start=True, stop=True
                )

                zb = wpool.tile([P, K * O], f32, tag="zb")
                nc.vector.tensor_tensor(
                    out=zb[:], in0=z_ps[:], in1=bias_sb[:], op=add
                )
                zr = wpool.tile([P, K * O], f32, tag="zr")
                nc.vector.tensor_scalar_max(out=zr[:], in0=zb[:], scalar1=0.0)
                o64 = wpool.tile([P, O], f32, tag="o64")
                nc.vector.tensor_tensor(
                    out=o64[:], in0=zr[:, 0:O], in1=zr[:, O : 2 * O], op=add
                )
                rep = wpool.tile([P, NROT, O], f32, tag="rep")
                nc.vector.tensor_copy(
                    out=rep[:],
                    in_=o64[:]
                    .rearrange("p (r o) -> p r o", r=1)
                    .to_broadcast((P, NROT, O)),
                )
                nc.sync.dma_start(out=out[t * P : (t + 1) * P, :, :], in_=rep[:])

    nc.compile()
    return nc


def get_module(tpc=TPC, loops=1, variant="full"):
    key = (tpc, loops, variant)
    if key not in _CACHE:
        _CACHE[key] = _build_module(tpc, loops, variant)
    return _CACHE[key]


def prep_inputs(mesh_signal, bary_coordinates, interp_coef, kernel_weights, bias):
    """Host-side marshalling: shard + resolve the irregular row lookup."""
    mesh_signal = np.ascontiguousarray(np.asarray(mesh_signal, dtype=np.float32))
    bary = np.asarray(bary_coordinates, dtype=np.float32)
    interp_coef = np.asarray(interp_coef, dtype=np.float32)
    kernel_weights = np.asarray(kernel_weights, dtype=np.float32)
    bias = np.asarray(bias, dtype=np.float32)

    idx_all = bary[..., 0].reshape(N, T).astype(np.int32)
    bw_all = np.ascontiguousarray(bary[..., 1].reshape(N, T))

    csum = interp_coef.sum(axis=(0, 1))  # (40,)
    cs3 = np.ascontiguousarray(
        np.broadcast_to(csum[:, None], (R * A, 3)).reshape(1, T)
    )
    wfc = np.ascontiguousarray(kernel_weights.transpose(2, 0, 1).reshape(F, K * O))
    bias2 = np.ascontiguousarray(bias.reshape(1, K * O))
    eye = np.eye(P, dtype=np.float32)

    mesh_t = np.ascontiguousarray(mesh_signal.T)  # (F, N)

    in_maps = []
    for c in range(NCORES):
        sl = slice(c * NV, (c + 1) * NV)
        idx_c = np.zeros((NVPAD, T), np.int32)
        idx_c[:NV] = idx_all[sl]
        bw_c = np.zeros((NVPAD, T), np.float32)
        bw_c[:NV] = bw_all[sl]

        # host-side row lookup, feature-major: gw[n, f, t] = mesh[idx[n,t], f]
        g_ft = mesh_t[:, idx_c]                     # (F, NVPAD, T)
        gw = np.ascontiguousarray(np.moveaxis(g_ft, 0, 1)).reshape(TPC, P, F, T)

        in_maps.append(
            {
                "gw": gw,
                "bwt": bw_c.reshape(TPC, P, T),
                "cs3": cs3,
                "wfc": wfc,
                "biasr": bias2,
                "ident": eye,
            }
        )
    return in_maps


def kernel(mesh_signal, bary_coordinates, interp_coef, kernel_weights, bias):
    from concourse.bass_utils import run_bass_kernel_spmd

    nc = get_module()
    in_maps = prep_inputs(
        mesh_signal, bary_coordinates, interp_coef, kernel_weights, bias
    )
    res = run_bass_kernel_spmd(nc, in_maps, list(range(NCORES))).results

    out = np.empty((N, NROT, O), np.float32)
    for c in range(NCORES):
        out[c * NV : (c + 1) * NV] = res[c]["out"][:NV]
    return out


# revision 8
# speedup vs baseline: 2.4544x; 2.4544x over previous
"""Trainium2 Bass kernel for nn_ConvIntrinsicLite.

Math notes
----------
The reference computes, per vertex n:
  pts[n,k,f]   = sum_j bw[n,k,j] * mesh[idx[n,k,j], f]          (k in 0..39 = (r,a))
  interp       = einsum('nkf,rak->nraf', pts, coef)
  out[n,s,:]   = sum_K relu( W @ sum_{r,a} roll(interp, s, a) + bias )

A circular roll along `a` followed by the full sum over (r,a) is roll-invariant,
so every rotation s yields identical values.  The (r,a) sum of interp collapses
onto csum[k] = sum_{r,a} coef[r,a,k]:
  S[n,f] = sum_{t=(k,j)} csum[k]*bw[n,t] * mesh[idx[n,t], f]    (120 terms)
  z[n,(K,o)] = S[n,:] @ Wfc + bias ;  out0 = relu(z)[:, :64] + relu(z)[:, 64:]
  out[n,s,:] = out0[n,:] for all 8 rotations.

Implementation split
--------------------
The environment's data-dependent DMA paths (anthropic extended dma_gather
ucode, multi-offset indirect DMA) are broken/disabled here, so the host
resolves the irregular row lookup G[n,t,:] = mesh_signal[idx[n,t],:] (pure
data movement).  All arithmetic runs on device, sharded over vertices on
8 cores:
  DVE: v = csum*bw, prod = G*v, S = sum_t prod
  PE : S^T via identity transpose, z = S @ Wfc
  DVE: +bias, relu, fold K pair, replicate 8 rotations; DMA out.
"""

import sys

import numpy as np

for _p in ("/opt/trn_rl_repo",):
    if _p not in sys.path:
        sys.path.insert(0, _p)

# problem dims
N, R, A, F = 25000, 5, 8, 32
K, O = 2, 64
NROT = 8            # rotations = range(0, A, 1)
T = R * A * 3       # 120 weighted terms per vertex
NCORES = 8
P = 128
NV = N // NCORES            # 3125 vertices per core
TPC = (NV + P - 1) // P     # 25 tiles per core
NVPAD = TPC * P             # 3200

_CACHE = {}


def _build_module(tpc, loops=1, variant="full"):
    import concourse.bacc as bacc
    import concourse.mybir as mybir
    from concourse.tile import TileContext

    f32 = mybir.dt.float32

    nc = bacc.Bacc("TRN2", target_bir_lowering=False, debug=False)

    # gathered mesh rows, feature-major per vertex: gw[n, f, t]
    gw = nc.dram_tensor("gw", [tpc, P, F, T], f32, kind="ExternalInput")
    # barycentric weights bw[n, t]
    bwt = nc.dram_tensor("bwt", [tpc, P, T], f32, kind="ExternalInput")
    # csum[k] replicated over j: cs3[t] = csum[t // 3], as a [1, T] row
    cs3 = nc.dram_tensor("cs3", [1, T], f32, kind="ExternalInput")
    wfc = nc.dram_tensor("wfc", [F, K * O], f32, kind="ExternalInput")
    biasr = nc.dram_tensor("biasr", [1, K * O], f32, kind="ExternalInput")
    ident = nc.dram_tensor("ident", [P, P], f32, kind="ExternalInput")
    out = nc.dram_tensor("out", [tpc * P, NROT, O], f32, kind="ExternalOutput")

    mult = mybir.AluOpType.mult
    add = mybir.AluOpType.add

    with TileContext(nc) as tc:
        with (
            tc.tile_pool(name="const", bufs=1) as cpool,
            tc.tile_pool(name="io", bufs=3) as iopool,
            tc.tile_pool(name="gath", bufs=3) as gpool,
            tc.tile_pool(name="work", bufs=2) as wpool,
            tc.tile_pool(name="psum", bufs=2, space="PSUM") as ppool,
        ):
            wfc_sb = cpool.tile([F, K * O], f32)
            nc.sync.dma_start(out=wfc_sb[:], in_=wfc[:])
            bias_sb = cpool.tile([1, K * O], f32)
            nc.sync.dma_start(out=bias_sb[:], in_=biasr[:])
            ones_sb = cpool.tile([1, P], f32)
            nc.gpsimd.memset(ones_sb[:], 1.0)
            ident_sb = cpool.tile([P, P], f32)
            nc.sync.dma_start(out=ident_sb[:], in_=ident[:])
            cs_sb = cpool.tile([P, T], f32)
            nc.sync.dma_start(out=cs_sb[:], in_=cs3[:].to_broadcast((P, T)))

            import contextlib

            loop_ctx = (
                tc.For_i(0, loops, 1) if loops > 1 else contextlib.nullcontext()
            )
            with loop_ctx:
              for t in range(tpc):
                g_sb = gpool.tile([P, F, T], f32, tag="g")
                nc.sync.dma_start(out=g_sb[:], in_=gw[t])
                bw_sb = iopool.tile([P, T], f32, tag="bw")
                nc.sync.dma_start(out=bw_sb[:], in_=bwt[t])

                if variant == "dmaonly":
                    rep0 = wpool.tile([P, NROT, O], f32, tag="rep")
                    nc.vector.tensor_copy(
                        out=rep0[:],
                        in_=g_sb[:, 0, 0:O]
                        .rearrange("p (r o) -> p r o", r=1)
                        .to_broadcast((P, NROT, O)),
                    )
                    nc.sync.dma_start(
                        out=out[t * P : (t + 1) * P, :, :], in_=rep0[:]
                    )
                    continue

                # v[p, t] = bw[p, t] * csum3[t]
                v_sb = wpool.tile([P, T], f32, tag="v")
                nc.vector.tensor_tensor(
                    out=v_sb[:], in0=bw_sb[:], in1=cs_sb[:], op=mult
                )
                # prod[p, f, t] = g[p, f, t] * v[p, t]
                prod = wpool.tile([P, F, T], f32, tag="prod")
                nc.vector.tensor_tensor(
                    out=prod[:],
                    in0=g_sb[:],
                    in1=v_sb[:]
                    .rearrange("p (o t) -> p o t", o=1)
                    .to_broadcast((P, F, T)),
                    op=mult,
                )
                # S[p, f] = sum_t prod[p, f, t]
                s_sb = wpool.tile([P, F], f32, tag="s")
                nc.vector.tensor_reduce(
                    out=s_sb[:], in_=prod[:], axis=mybir.AxisListType.X, op=add
                )

                st_ps = ppool.tile([F, P], f32, tag="stp")
                nc.tensor.transpose(out=st_ps[:], in_=s_sb[:], identity=ident_sb[:])
                st_sb = wpool.tile([F, P], f32, tag="st")
                nc.scalar.copy(out=st_sb[:], in_=st_ps[:])

                # z = S @ Wfc + 1s.T @ bias  (bias folded into the PSUM group)
                z_ps = ppool.tile([P, K * O], f32, tag="z")
                nc.tensor.matmul(
                    out=z_ps[:], lhsT=st_sb[:], rhs=wfc_sb[:], start=True, stop=False
                )
                nc.tensor.matmul(
                    out=z_ps[:], lhsT=ones_sb[:], rhs=bias_sb[:],
                    start=False, stop=True,
                )

                zr = wpool.tile([P, K * O], f32, tag="zr")
                nc.scalar.activation(
                    out=zr[:], in_=z_ps[:],
                    func=mybir.ActivationFunctionType.Relu,
                )
                o64 = wpool.tile([P, O], f32, tag="o64")
                nc.vector.tensor_tensor(
                    out=o64[:], in0=zr[:, 0:O], in1=zr[:, O : 2 * O], op=add
                )
                rep = wpool.tile([P, NROT, O], f32, tag="rep")
                nc.scalar.copy(
                    out=rep[:],
                    in_=o64[:]
                    .rearrange("p (r o) -> p r o", r=1)
                    .to_broadcast((P, NROT, O)),
                )
                nc.sync.dma_start(out=out[t * P : (t + 1) * P, :, :], in_=rep[:])

    nc.compile()
    return nc


def get_module(tpc=TPC, loops=1, variant="full"):
    key = (tpc, loops, variant)
    if key not in _CACHE:
        _CACHE[key] = _build_module(tpc, loops, variant)
    return _CACHE[key]


def prep_inputs(mesh_signal, bary_coordinates, interp_coef, kernel_weights, bias):
    """Host-side marshalling: shard + resolve the irregular row lookup."""
    mesh_signal = np.ascontiguousarray(np.asarray(mesh_signal, dtype=np.float32))
    bary = np.asarray(bary_coordinates, dtype=np.float32)
    interp_coef = np.asarray(interp_coef, dtype=np.float32)
    kernel_weights = np.asarray(kernel_weights, dtype=np.float32)
    bias = np.asarray(bias, dtype=np.float32)

    idx_all = bary[..., 0].reshape(N, T).astype(np.int32)
    bw_all = np.ascontiguousarray(bary[..., 1].reshape(N, T))

    csum = interp_coef.sum(axis=(0, 1))  # (40,)
    cs3 = np.ascontiguousarray(
        np.broadcast_to(csum[:, None], (R * A, 3)).reshape(1, T)
    )
    wfc = np.ascontiguousarray(kernel_weights.transpose(2, 0, 1).reshape(F, K * O))
    bias2 = np.ascontiguousarray(bias.reshape(1, K * O))
    eye = np.eye(P, dtype=np.float32)

    mesh_t = np.ascontiguousarray(mesh_signal.T)  # (F, N)

    in_maps = []
    for c in range(NCORES):
        sl = slice(c * NV, (c + 1) * NV)
        idx_c = np.zeros((NVPAD, T), np.int32)
        idx_c[:NV] = idx_all[sl]
        bw_c = np.zeros((NVPAD, T), np.float32)
        bw_c[:NV] = bw_all[sl]

        # host-side row lookup, feature-major: gw[n, f, t] = mesh[idx[n,t], f]
        g_ft = mesh_t[:, idx_c]                     # (F, NVPAD, T)
        gw = np.ascontiguousarray(np.moveaxis(g_ft, 0, 1)).reshape(TPC, P, F, T)

        in_maps.append(
            {
                "gw": gw,
                "bwt": bw_c.reshape(TPC, P, T),
                "cs3": cs3,
                "wfc": wfc,
                "biasr": bias2,
                "ident": eye,
            }
        )
    return in_maps


def kernel(mesh_signal, bary_coordinates, interp_coef, kernel_weights, bias):
    from concourse.bass_utils import run_bass_kernel_spmd

    nc = get_module()
    in_maps = prep_inputs(
        mesh_signal, bary_coordinates, interp_coef, kernel_weights, bias
    )
    res = run_bass_kernel_spmd(nc, in_maps, list(range(NCORES))).results

    out = np.empty((N, NROT, O), np.float32)
    for c in range(NCORES):
        out[c * NV : (c + 1) * NV] = res[c]["out"][:NV]
    return out
